# revision 14
# baseline (speedup 1.0000x reference)
"""Grouped SwiGLU MoE expert FFN on 8 Trainium2 NeuronCores.

Problem: out[t] = w2[e(t)] @ (silu(w1[e(t)] x[t]) * (w3[e(t)] x[t])),
T=4096 tokens sorted by expert, E=8 experts, H=1024, I=2816, fp32.

Strategy (expert-parallel + token balancing, no collectives):
  - Token groups (per expert, multiples of 128) are decomposed into eight
    384-token pieces and eight 128-token pieces; each core gets exactly one
    of each (512 tokens), so compute is perfectly balanced across cores.
  - Each core's SPMD program has two weight slots (the two pieces'
    experts). Weights are streamed from HBM once per slot.
  - Matmul layout keeps weights stationary and tokens moving:
      u^T[i] = sum_k w1[k,i]^T x^T[k,:]   (PSUM, accumulate over H tiles)
      h^T[i] = silu(u^T[i]) * v^T[i]      (ACT + DVE)
      out^T[m] = sum_i w2[i,m]^T h^T[i]   (PSUM, accumulate over I tiles)
    x^T / weight blocks are pre-laid-out on the host so every DMA is
    fully contiguous per partition.
  - dtype variants (MOE_VARIANT env, default fp16):
      "fp16": weights/activations cast to fp16 (values are O(1), fp16-safe);
              ~129 us/core model time, ~5e-4 rel-L2 vs the fp32 reference.
      "f32r": fp32 bits at full PE rate via float32r; exactness-oriented
              (~2.6e-4), DMA-bound ~230 us. f32r needs >=256 moving rows, so
              the 128-piece is computed as rows [256:512] overlapping the
              384-piece and the duplicated rows are discarded on the host.
      "bf16": like fp16 but bf16 (~4e-3) — dominated by fp16, kept for tests.

Self-contained: only needs numpy + the concourse/axon runtime.
"""

import numpy as np
import ml_dtypes

import jax
import concourse.tile as tile
from concourse import bacc, mybir

E, H, II = 8, 1024, 2816
NCORES = 8
TOK = 512            # tokens per core
KT = H // 128        # 8  contraction tiles (stage 1)
IT = II // 128       # 22 intermediate tiles
MT = H // 128        # 8  output tiles

F32 = mybir.dt.float32
F32R = mybir.dt.float32r
BF16 = mybir.dt.bfloat16
FP16 = mybir.dt.float16

import os
VARIANT = os.environ.get("MOE_VARIANT", "fp16")  # "f32r" | "bf16"


# ----------------------------------------------------------------------------
# Planning: decompose ragged groups into eight 384-pieces + eight 128-pieces.
# ----------------------------------------------------------------------------

def _plan_pieces(group_sizes):
    """Return (pieces384, pieces128): lists of (expert, tok_start), 8 each.

    Requires all group sizes to be multiples of 128 and sum to 4096.
    """
    g = [int(v) for v in group_sizes]
    if sum(g) != NCORES * TOK or any(v % 128 for v in g) or len(g) != E:
        return None
    offs = np.concatenate([[0], np.cumsum(g)])
    # x_e = number of 384-pieces for expert e; need sum(x)=8, 384*x_e <= g_e.
    x = [v // 384 for v in g]
    total = sum(x)
    if total < 8:
        return None
    e_i = 0
    while total > 8:  # shed surplus 384-pieces (each becomes three 128s)
        if x[e_i] > 0:
            x[e_i] -= 1
            total -= 1
        e_i = (e_i + 1) % E
    p384, p128 = [], []
    for e in range(E):
        t = int(offs[e])
        for _ in range(x[e]):
            p384.append((e, t))
            t += 384
        while t < offs[e + 1]:
            p128.append((e, t))
            t += 128
    if len(p384) != 8 or len(p128) != 8:
        return None
    return p384, p128


# ----------------------------------------------------------------------------
# Device program
# ----------------------------------------------------------------------------

def _build_program(chunks, wdt, hdt, out_cols):
    """chunks: list of (row0, nrows, slot). out_cols[j] = cols of output j.

    Schedule: both chunks' phase-1 first (interleaved weight streaming),
    then both phase-2s; buffer counts tuned via TimelineSim.
    """
    nc = bacc.Bacc()
    nslots = 1 + max(s for _, _, s in chunks)
    nchunks = len(chunks)

    xT = nc.declare_dram_parameter("xT", [128, KT, TOK], wdt, isOutput=False)
    wp = {}
    for s in range(nslots):
        wp[s, "w1"] = nc.declare_dram_parameter(f"w1_{s}", [IT, 128, KT * 128], wdt, isOutput=False)
        wp[s, "w3"] = nc.declare_dram_parameter(f"w3_{s}", [IT, 128, KT * 128], wdt, isOutput=False)
        wp[s, "w2"] = nc.declare_dram_parameter(f"w2_{s}", [IT, 128, MT * 128], wdt, isOutput=False)
    outs = [
        nc.declare_dram_parameter(f"outT_{j}", [MT, 128, c], F32, isOutput=True)
        for j, c in enumerate(out_cols)
    ]

    is_fp32 = wdt == F32R
    # fp32 tiles are 2x the size; shrink pools to fit SBUF
    w13_bufs = 4 if is_fp32 else 6
    w2_bufs = nchunks * IT + 1 if not is_fp32 else IT + 1
    h_bufs = nchunks * IT + 2 if not is_fp32 else IT + 2
    interleave = not is_fp32

    with tile.TileContext(nc) as tc:
        with tc.tile_pool(name="xp", bufs=1) as xp, \
             tc.tile_pool(name="w13", bufs=w13_bufs) as w13p, \
             tc.tile_pool(name="w2", bufs=w2_bufs) as w2p, \
             tc.tile_pool(name="h", bufs=h_bufs) as hp, \
             tc.tile_pool(name="su", bufs=4) as sup, \
             tc.tile_pool(name="oc", bufs=6) as ocp, \
             tc.tile_pool(name="up", bufs=3, space="PSUM") as up, \
             tc.tile_pool(name="vp", bufs=3, space="PSUM") as vp, \
             tc.tile_pool(name="op", bufs=2, space="PSUM") as op:

            warm = sup.tile([1, 16], wdt, tag="warm")
            nc.vector.memset(warm[:], 0.0)
            wps = op.tile([16, 16], F32, tag="o", name="warmps")
            for _ in range(64):
                nc.tensor.matmul(wps[:], warm[:], warm[:], start=True, stop=True)

            xt = xp.tile([128, KT, TOK], wdt)
            nc.sync.dma_start(out=xt[:], in_=xT[:])

            w2ts = {}
            hts = {}

            def phase1(ci):
                r0, nr, s = chunks[ci]
                w2ts[ci] = []
                hts[ci] = []
                for i in range(IT):
                    w1t = w13p.tile([128, KT * 128], wdt, tag="w1")
                    nc.sync.dma_start(out=w1t[:], in_=wp[s, "w1"][i])
                    w3t = w13p.tile([128, KT * 128], wdt, tag="w3")
                    nc.sync.dma_start(out=w3t[:], in_=wp[s, "w3"][i])
                    w2t = w2p.tile([128, MT * 128], wdt, tag="w2")
                    nc.sync.dma_start(out=w2t[:], in_=wp[s, "w2"][i])
                    w2ts[ci].append(w2t)

                    u = up.tile([128, nr], F32, tag="u")
                    v = vp.tile([128, nr], F32, tag="v")
                    w1r = w1t[:].rearrange("p (k j) -> p k j", k=KT)
                    w3r = w3t[:].rearrange("p (k j) -> p k j", k=KT)
                    for k in range(KT):
                        nc.tensor.matmul(
                            u[:], w1r[:, k, :], xt[:, k, r0:r0 + nr],
                            start=(k == 0), stop=(k == KT - 1),
                        )
                    for k in range(KT):
                        nc.tensor.matmul(
                            v[:], w3r[:, k, :], xt[:, k, r0:r0 + nr],
                            start=(k == 0), stop=(k == KT - 1),
                        )
                    su = sup.tile([128, nr], F32, tag="su")
                    nc.scalar.activation(
                        out=su[:], in_=u[:],
                        func=mybir.ActivationFunctionType.Silu,
                    )
                    ht = hp.tile([128, nr], hdt, tag="h")
                    nc.vector.tensor_mul(ht[:], su[:], v[:])
                    hts[ci].append(ht)

            def phase2(ci):
                r0, nr, s = chunks[ci]
                for m in range(MT):
                    o = op.tile([128, nr], F32, tag="o")
                    for i in range(IT):
                        w2r = w2ts[ci][i][:].rearrange("p (m j) -> p m j", m=MT)
                        nc.tensor.matmul(
                            o[:], w2r[:, m, :], hts[ci][i][:],
                            start=(i == 0), stop=(i == IT - 1),
                        )
                    oc = ocp.tile([128, nr], F32, tag="oc")
                    nc.scalar.copy(out=oc[:], in_=o[:])
                    nc.sync.dma_start(out=outs[ci][m], in_=oc[:])

            if interleave:
                for ci in range(nchunks):
                    phase1(ci)
                for ci in range(nchunks):
                    phase2(ci)
            else:
                for ci in range(nchunks):
                    phase1(ci)
                    phase2(ci)

    nc.finalize()
    return nc


# ----------------------------------------------------------------------------
# Host-side data prep
# ----------------------------------------------------------------------------

def _np_dtype(variant):
    if variant == "f32r":
        return np.float32
    return np.float16 if variant == "fp16" else ml_dtypes.bfloat16


def _fmt_w13(w, dt):
    # [H, I] -> [IT, 128, KT*128]; block i, partition p, col k*128+j = w[k*128+p, i*128+j]
    return np.ascontiguousarray(
        w.reshape(KT, 128, IT, 128).transpose(2, 1, 0, 3).reshape(IT, 128, KT * 128)
    ).astype(dt)


def _fmt_w2(w, dt):
    # [I, H] -> [IT, 128, MT*128] (already contiguous blocks)
    return np.ascontiguousarray(w.reshape(IT, 128, MT * 128)).astype(dt)


def _fmt_xT(x, dt):
    # [TOK, H] -> [128, KT, TOK]; partition p, k, t = x[t, k*128+p]
    return np.ascontiguousarray(x.T.reshape(KT, 128, TOK).transpose(1, 0, 2)).astype(dt)


_CACHE = {}


def _get_runner(variant, gs_key, group_sizes):
    """Build (or fetch) the compiled SPMD runner for these group sizes."""
    key = (variant, gs_key)
    if key in _CACHE:
        return _CACHE[key]

    plan = _plan_pieces(group_sizes)
    if plan is None:
        raise NotImplementedError(
            f"group_sizes {list(group_sizes)} not decomposable into 384/128 pieces"
        )
    p384, p128 = plan

    if variant == "f32r":
        wdt = hdt = F32R
        chunks = [(0, 384, 0), (256, 256, 1)]
        out_cols = [384, 256]
        bcol0 = 128  # cols of chunk-1 output corresponding to the 128-piece
    else:
        wdt = hdt = FP16 if variant == "fp16" else BF16
        chunks = [(0, 384, 0), (384, 128, 1)]
        out_cols = [384, 128]
        bcol0 = 0

    nc = _build_program(chunks, wdt, hdt, out_cols)
    runner = _make_pjrt_runner(nc)
    st = {
        "nc": nc, "runner": runner, "p384": p384, "p128": p128,
        "variant": variant, "bcol0": bcol0,
    }
    _CACHE[key] = st
    return st


def _make_pjrt_runner(nc):
    """Persistent jit'd SPMD executor (mirrors bass2jax.run_bass_via_pjrt)."""
    from jax.sharding import Mesh, PartitionSpec
    from jax.experimental.shard_map import shard_map
    from concourse.bass2jax import (
        _bass_exec_p, install_neuronx_cc_hook, partition_id_tensor,
    )

    install_neuronx_cc_hook()

    partition_name = nc.partition_id_tensor.name if nc.partition_id_tensor else None
    in_names, out_names, out_avals = [], [], []
    for alloc in nc.m.functions[0].allocations:
        if not isinstance(alloc, mybir.MemoryLocationSet):
            continue
        name = alloc.memorylocations[0].name
        if alloc.kind == "ExternalInput":
            if name != partition_name:
                in_names.append(name)
        elif alloc.kind == "ExternalOutput":
            out_names.append(name)
            out_avals.append(
                jax.core.ShapedArray(tuple(alloc.tensor_shape), mybir.dt.np(alloc.dtype))
            )
    n_params = len(in_names)
    n_outs = len(out_names)
    all_in_names = list(in_names) + list(out_names)
    if partition_name is not None:
        all_in_names.append(partition_name)
    donate = tuple(range(n_params, n_params + n_outs))

    def _body(*args):
        operands = list(args)
        if partition_name is not None:
            operands.append(partition_id_tensor())
        outs = _bass_exec_p.bind(
            *operands,
            out_avals=tuple(out_avals),
            in_names=tuple(all_in_names),
            out_names=tuple(out_names),
            lowering_input_output_aliases=(),
            sim_require_finite=True,
            sim_require_nnan=True,
            nc=nc,
        )
        return tuple(outs)

    devices = jax.devices()[:NCORES]
    mesh = Mesh(np.asarray(devices), ("core",))
    in_specs = (PartitionSpec("core"),) * (n_params + n_outs)
    out_specs = (PartitionSpec("core"),) * n_outs
    jitted = jax.jit(
        shard_map(_body, mesh=mesh, in_specs=in_specs, out_specs=out_specs,
                  check_rep=False),
        donate_argnums=donate, keep_unused=True,
    )

    def run(in_maps):
        per_core = [[np.asarray(m[n]) for n in in_names] for m in in_maps]
        concat_in = [
            np.concatenate([per_core[c][i] for c in range(NCORES)], axis=0)
            for i in range(n_params)
        ]
        zeros = [
            np.zeros((NCORES * a.shape[0], *a.shape[1:]), a.dtype) for a in out_avals
        ]
        out_arrs = jitted(*concat_in, *zeros)
        return [
            {
                name: np.asarray(out_arrs[i]).reshape(NCORES, *out_avals[i].shape)[c]
                for i, name in enumerate(out_names)
            }
            for c in range(NCORES)
        ]

    return run


def _prep_in_maps(st, hidden_states, w1, w2, w3):
    dt = _np_dtype(st["variant"])
    w1f = [_fmt_w13(np.asarray(w1[e]), dt) for e in range(E)]
    w3f = [_fmt_w13(np.asarray(w3[e]), dt) for e in range(E)]
    w2f = [_fmt_w2(np.asarray(w2[e]), dt) for e in range(E)]
    hs = np.asarray(hidden_states)

    in_maps = []
    for c in range(NCORES):
        eA, tA = st["p384"][c]
        eB, tB = st["p128"][c]
        xc = np.concatenate([hs[tA:tA + 384], hs[tB:tB + 128]], axis=0)
        in_maps.append({
            "xT": _fmt_xT(xc, dt),
            "w1_0": w1f[eA], "w3_0": w3f[eA], "w2_0": w2f[eA],
            "w1_1": w1f[eB], "w3_1": w3f[eB], "w2_1": w2f[eB],
        })
    return in_maps


def _assemble(st, results, out_dtype):
    out = np.empty((NCORES * TOK, H), dtype=out_dtype)
    bc = st["bcol0"]
    for c in range(NCORES):
        eA, tA = st["p384"][c]
        eB, tB = st["p128"][c]
        oA = results[c]["outT_0"].reshape(H, 384)   # [MT,128,384] -> [H,384]
        oB = results[c]["outT_1"]
        out[tA:tA + 384] = oA.T
        if oB.shape == (128, H):                    # token-major fast path
            out[tB:tB + 128] = oB
        else:
            oB = oB.reshape(H, oB.shape[-1])
            out[tB:tB + 128] = oB[:, bc:bc + 128].T
    return out


def kernel(hidden_states, group_sizes, w1, w2, w3):
    gs = np.asarray(group_sizes)
    st = _get_runner(VARIANT, gs.tobytes(), gs)
    in_maps = _prep_in_maps(st, hidden_states, w1, w2, w3)
    results = st["runner"](in_maps)
    return _assemble(st, results, np.asarray(hidden_states).dtype)


# revision 21
# speedup vs baseline: 1.0017x; 1.0017x over previous
"""Grouped SwiGLU MoE expert FFN on 8 Trainium2 NeuronCores.

Problem: out[t] = w2[e(t)] @ (silu(w1[e(t)] x[t]) * (w3[e(t)] x[t])),
T=4096 tokens sorted by expert, E=8 experts, H=1024, I=2816, fp32.

Strategy (expert-parallel + token balancing, no collectives):
  - Token groups (per expert, multiples of 128) are decomposed into eight
    384-token pieces and eight 128-token pieces; each core gets exactly one
    of each (512 tokens), so compute is perfectly balanced across cores.
  - Each core's SPMD program has two weight slots (the two pieces'
    experts). Weights are streamed from HBM once per slot.
  - Matmul layout keeps weights stationary and tokens moving:
      u^T[i] = sum_k w1[k,i]^T x^T[k,:]   (PSUM, accumulate over H tiles)
      h^T[i] = silu(u^T[i]) * v^T[i]      (ACT + DVE)
      out^T[m] = sum_i w2[i,m]^T h^T[i]   (PSUM, accumulate over I tiles)
    x^T / weight blocks are pre-laid-out on the host so every DMA is
    fully contiguous per partition.
  - dtype variants (MOE_VARIANT env, default fp16):
      "fp16": weights/activations cast to fp16 (values are O(1), fp16-safe);
              ~129 us/core model time, ~5e-4 rel-L2 vs the fp32 reference.
      "f32r": fp32 bits at full PE rate via float32r; exactness-oriented
              (~2.6e-4), DMA-bound ~230 us. f32r needs >=256 moving rows, so
              the 128-piece is computed as rows [256:512] overlapping the
              384-piece and the duplicated rows are discarded on the host.
      "bf16": like fp16 but bf16 (~4e-3) — dominated by fp16, kept for tests.

Self-contained: only needs numpy + the concourse/axon runtime.
"""

import numpy as np
import ml_dtypes

import jax
import concourse.tile as tile
from concourse import bacc, mybir

E, H, II = 8, 1024, 2816
NCORES = 8
TOK = 512            # tokens per core
KT = H // 128        # 8  contraction tiles (stage 1)
IT = II // 128       # 22 intermediate tiles
MT = H // 128        # 8  output tiles

F32 = mybir.dt.float32
F32R = mybir.dt.float32r
BF16 = mybir.dt.bfloat16
FP16 = mybir.dt.float16

import os
VARIANT = os.environ.get("MOE_VARIANT", "fp16")  # "f32r" | "bf16"


# ----------------------------------------------------------------------------
# Planning: decompose ragged groups into eight 384-pieces + eight 128-pieces.
# ----------------------------------------------------------------------------

def _plan_pieces(group_sizes):
    """Return (pieces384, pieces128): lists of (expert, tok_start), 8 each.

    Requires all group sizes to be multiples of 128 and sum to 4096.
    """
    g = [int(v) for v in group_sizes]
    if sum(g) != NCORES * TOK or any(v % 128 for v in g) or len(g) != E:
        return None
    offs = np.concatenate([[0], np.cumsum(g)])
    # x_e = number of 384-pieces for expert e; need sum(x)=8, 384*x_e <= g_e.
    x = [v // 384 for v in g]
    total = sum(x)
    if total < 8:
        return None
    e_i = 0
    while total > 8:  # shed surplus 384-pieces (each becomes three 128s)
        if x[e_i] > 0:
            x[e_i] -= 1
            total -= 1
        e_i = (e_i + 1) % E
    p384, p128 = [], []
    for e in range(E):
        t = int(offs[e])
        for _ in range(x[e]):
            p384.append((e, t))
            t += 384
        while t < offs[e + 1]:
            p128.append((e, t))
            t += 128
    if len(p384) != 8 or len(p128) != 8:
        return None
    return p384, p128


# ----------------------------------------------------------------------------
# Device program
# ----------------------------------------------------------------------------

def _build_program(chunks, wdt, hdt, out_cols):
    """chunks: list of (row0, nrows, slot). out_cols[j] = cols of output j.

    Schedule: both chunks' phase-1 first (interleaved weight streaming),
    then both phase-2s; buffer counts tuned via TimelineSim.
    """
    nc = bacc.Bacc()
    nslots = 1 + max(s for _, _, s in chunks)
    nchunks = len(chunks)

    xTs = [
        nc.declare_dram_parameter(f"xT_{j}", [128, KT, nr], wdt, isOutput=False)
        for j, (_, nr, _) in enumerate(chunks)
    ]
    wp = {}
    for s in range(nslots):
        wp[s, "w1"] = nc.declare_dram_parameter(f"w1_{s}", [IT, 128, KT * 128], wdt, isOutput=False)
        wp[s, "w3"] = nc.declare_dram_parameter(f"w3_{s}", [IT, 128, KT * 128], wdt, isOutput=False)
        wp[s, "w2"] = nc.declare_dram_parameter(f"w2_{s}", [IT, 128, MT * 128], wdt, isOutput=False)
    outs = [
        nc.declare_dram_parameter(f"outT_{j}", [MT, 128, c], F32, isOutput=True)
        for j, c in enumerate(out_cols)
    ]

    is_fp32 = wdt == F32R
    # fp32 tiles are 2x the size; shrink pools to fit SBUF
    w13_bufs = 4 if is_fp32 else 6
    w2_bufs = nchunks * IT + 1 if not is_fp32 else IT + 1
    h_bufs = nchunks * IT + 2 if not is_fp32 else IT + 2
    interleave = not is_fp32

    with tile.TileContext(nc) as tc:
        with tc.tile_pool(name="xp", bufs=1) as xp, \
             tc.tile_pool(name="w13", bufs=w13_bufs) as w13p, \
             tc.tile_pool(name="w2", bufs=w2_bufs) as w2p, \
             tc.tile_pool(name="h", bufs=h_bufs) as hp, \
             tc.tile_pool(name="su", bufs=4) as sup, \
             tc.tile_pool(name="oc", bufs=6) as ocp, \
             tc.tile_pool(name="up", bufs=3, space="PSUM") as up, \
             tc.tile_pool(name="vp", bufs=3, space="PSUM") as vp, \
             tc.tile_pool(name="op", bufs=2, space="PSUM") as op:

            warm = sup.tile([1, 16], wdt, tag="warm")
            nc.vector.memset(warm[:], 0.0)
            wps = op.tile([16, 16], F32, tag="o", name="warmps")
            for _ in range(64):
                nc.tensor.matmul(wps[:], warm[:], warm[:], start=True, stop=True)

            xts = []
            for j, (_, nr, _) in enumerate(chunks):
                xt_j = xp.tile([128, KT, nr], wdt, name=f"xt{j}", tag=f"xt{j}")
                nc.sync.dma_start(out=xt_j[:], in_=xTs[j][:])
                xts.append(xt_j)

            w2ts = {}
            hts = {}

            def phase1(ci):
                r0, nr, s = chunks[ci]
                w2ts[ci] = []
                hts[ci] = []
                for i in range(IT):
                    w1t = w13p.tile([128, KT * 128], wdt, tag="w1")
                    nc.sync.dma_start(out=w1t[:], in_=wp[s, "w1"][i])
                    w3t = w13p.tile([128, KT * 128], wdt, tag="w3")
                    nc.sync.dma_start(out=w3t[:], in_=wp[s, "w3"][i])
                    w2t = w2p.tile([128, MT * 128], wdt, tag="w2")
                    nc.sync.dma_start(out=w2t[:], in_=wp[s, "w2"][i])
                    w2ts[ci].append(w2t)

                    u = up.tile([128, nr], F32, tag="u")
                    v = vp.tile([128, nr], F32, tag="v")
                    w1r = w1t[:].rearrange("p (k j) -> p k j", k=KT)
                    w3r = w3t[:].rearrange("p (k j) -> p k j", k=KT)
                    for k in range(KT):
                        nc.tensor.matmul(
                            u[:], w1r[:, k, :], xts[ci][:, k, :],
                            start=(k == 0), stop=(k == KT - 1),
                        )
                    for k in range(KT):
                        nc.tensor.matmul(
                            v[:], w3r[:, k, :], xts[ci][:, k, :],
                            start=(k == 0), stop=(k == KT - 1),
                        )
                    su = sup.tile([128, nr], F32, tag="su")
                    nc.scalar.activation(
                        out=su[:], in_=u[:],
                        func=mybir.ActivationFunctionType.Silu,
                    )
                    ht = hp.tile([128, nr], hdt, tag="h")
                    nc.vector.tensor_mul(ht[:], su[:], v[:])
                    hts[ci].append(ht)

            def phase2(ci):
                r0, nr, s = chunks[ci]
                for m in range(MT):
                    o = op.tile([128, nr], F32, tag="o")
                    for i in range(IT):
                        w2r = w2ts[ci][i][:].rearrange("p (m j) -> p m j", m=MT)
                        nc.tensor.matmul(
                            o[:], w2r[:, m, :], hts[ci][i][:],
                            start=(i == 0), stop=(i == IT - 1),
                        )
                    oc = ocp.tile([128, nr], F32, tag="oc")
                    nc.vector.tensor_copy(out=oc[:], in_=o[:])
                    nc.sync.dma_start(out=outs[ci][m], in_=oc[:])

            if interleave:
                for ci in range(nchunks):
                    phase1(ci)
                for ci in range(nchunks):
                    phase2(ci)
            else:
                for ci in range(nchunks):
                    phase1(ci)
                    phase2(ci)

    nc.finalize()
    return nc


# ----------------------------------------------------------------------------
# Host-side data prep
# ----------------------------------------------------------------------------

def _np_dtype(variant):
    if variant == "f32r":
        return np.float32
    return np.float16 if variant == "fp16" else ml_dtypes.bfloat16


def _fmt_w13(w, dt):
    # [H, I] -> [IT, 128, KT*128]; block i, partition p, col k*128+j = w[k*128+p, i*128+j]
    return np.ascontiguousarray(
        w.reshape(KT, 128, IT, 128).transpose(2, 1, 0, 3).reshape(IT, 128, KT * 128)
    ).astype(dt)


def _fmt_w2(w, dt):
    # [I, H] -> [IT, 128, MT*128] (already contiguous blocks)
    return np.ascontiguousarray(w.reshape(IT, 128, MT * 128)).astype(dt)


def _fmt_xT(x, dt):
    # [n, H] -> [128, KT, n]; partition p, k, t = x[t, k*128+p]
    n = x.shape[0]
    return np.ascontiguousarray(x.T.reshape(KT, 128, n).transpose(1, 0, 2)).astype(dt)


_CACHE = {}


def _get_runner(variant, gs_key, group_sizes):
    """Build (or fetch) the compiled SPMD runner for these group sizes."""
    key = (variant, gs_key)
    if key in _CACHE:
        return _CACHE[key]

    plan = _plan_pieces(group_sizes)
    if plan is None:
        raise NotImplementedError(
            f"group_sizes {list(group_sizes)} not decomposable into 384/128 pieces"
        )
    p384, p128 = plan

    if variant == "f32r":
        wdt = hdt = F32R
        chunks = [(0, 384, 0), (256, 256, 1)]
        out_cols = [384, 256]
        bcol0 = 128  # cols of chunk-1 output corresponding to the 128-piece
    else:
        wdt = hdt = FP16 if variant == "fp16" else BF16
        chunks = [(0, 384, 0), (384, 128, 1)]
        out_cols = [384, 128]
        bcol0 = 0

    nc = _build_program(chunks, wdt, hdt, out_cols)
    runner = _make_pjrt_runner(nc)
    st = {
        "nc": nc, "runner": runner, "p384": p384, "p128": p128,
        "variant": variant, "bcol0": bcol0, "chunks": chunks,
    }
    _CACHE[key] = st
    return st


def _make_pjrt_runner(nc):
    """Persistent jit'd SPMD executor (mirrors bass2jax.run_bass_via_pjrt)."""
    from jax.sharding import Mesh, PartitionSpec
    from jax.experimental.shard_map import shard_map
    from concourse.bass2jax import (
        _bass_exec_p, install_neuronx_cc_hook, partition_id_tensor,
    )

    install_neuronx_cc_hook()

    partition_name = nc.partition_id_tensor.name if nc.partition_id_tensor else None
    in_names, out_names, out_avals = [], [], []
    for alloc in nc.m.functions[0].allocations:
        if not isinstance(alloc, mybir.MemoryLocationSet):
            continue
        name = alloc.memorylocations[0].name
        if alloc.kind == "ExternalInput":
            if name != partition_name:
                in_names.append(name)
        elif alloc.kind == "ExternalOutput":
            out_names.append(name)
            out_avals.append(
                jax.core.ShapedArray(tuple(alloc.tensor_shape), mybir.dt.np(alloc.dtype))
            )
    n_params = len(in_names)
    n_outs = len(out_names)
    all_in_names = list(in_names) + list(out_names)
    if partition_name is not None:
        all_in_names.append(partition_name)
    donate = tuple(range(n_params, n_params + n_outs))

    def _body(*args):
        operands = list(args)
        if partition_name is not None:
            operands.append(partition_id_tensor())
        outs = _bass_exec_p.bind(
            *operands,
            out_avals=tuple(out_avals),
            in_names=tuple(all_in_names),
            out_names=tuple(out_names),
            lowering_input_output_aliases=(),
            sim_require_finite=True,
            sim_require_nnan=True,
            nc=nc,
        )
        return tuple(outs)

    devices = jax.devices()[:NCORES]
    mesh = Mesh(np.asarray(devices), ("core",))
    in_specs = (PartitionSpec("core"),) * (n_params + n_outs)
    out_specs = (PartitionSpec("core"),) * n_outs
    jitted = jax.jit(
        shard_map(_body, mesh=mesh, in_specs=in_specs, out_specs=out_specs,
                  check_rep=False),
        donate_argnums=donate, keep_unused=True,
    )

    def run(in_maps):
        per_core = [[np.asarray(m[n]) for n in in_names] for m in in_maps]
        concat_in = [
            np.concatenate([per_core[c][i] for c in range(NCORES)], axis=0)
            for i in range(n_params)
        ]
        zeros = [
            np.zeros((NCORES * a.shape[0], *a.shape[1:]), a.dtype) for a in out_avals
        ]
        out_arrs = jitted(*concat_in, *zeros)
        return [
            {
                name: np.asarray(out_arrs[i]).reshape(NCORES, *out_avals[i].shape)[c]
                for i, name in enumerate(out_names)
            }
            for c in range(NCORES)
        ]

    return run


def _prep_in_maps(st, hidden_states, w1, w2, w3):
    dt = _np_dtype(st["variant"])
    w1f = [_fmt_w13(np.asarray(w1[e]), dt) for e in range(E)]
    w3f = [_fmt_w13(np.asarray(w3[e]), dt) for e in range(E)]
    w2f = [_fmt_w2(np.asarray(w2[e]), dt) for e in range(E)]
    hs = np.asarray(hidden_states)

    in_maps = []
    for c in range(NCORES):
        eA, tA = st["p384"][c]
        eB, tB = st["p128"][c]
        xc = np.concatenate([hs[tA:tA + 384], hs[tB:tB + 128]], axis=0)
        r1, n1 = st["chunks"][1][0], st["chunks"][1][1]
        in_maps.append({
            "xT_0": _fmt_xT(xc[0:384], dt),
            "xT_1": _fmt_xT(xc[r1:r1 + n1], dt),
            "w1_0": w1f[eA], "w3_0": w3f[eA], "w2_0": w2f[eA],
            "w1_1": w1f[eB], "w3_1": w3f[eB], "w2_1": w2f[eB],
        })
    return in_maps


def _assemble(st, results, out_dtype):
    out = np.empty((NCORES * TOK, H), dtype=out_dtype)
    bc = st["bcol0"]
    for c in range(NCORES):
        eA, tA = st["p384"][c]
        eB, tB = st["p128"][c]
        oA = results[c]["outT_0"].reshape(H, 384)   # [MT,128,384] -> [H,384]
        oB = results[c]["outT_1"]
        out[tA:tA + 384] = oA.T
        if oB.shape == (128, H):                    # token-major fast path
            out[tB:tB + 128] = oB
        else:
            oB = oB.reshape(H, oB.shape[-1])
            out[tB:tB + 128] = oB[:, bc:bc + 128].T
    return out


def kernel(hidden_states, group_sizes, w1, w2, w3):
    gs = np.asarray(group_sizes)
    st = _get_runner(VARIANT, gs.tobytes(), gs)
    in_maps = _prep_in_maps(st, hidden_states, w1, w2, w3)
    results = st["runner"](in_maps)
    return _assemble(st, results, np.asarray(hidden_states).dtype)
